# revision 40
# baseline (speedup 1.0000x reference)
"""BlockRadiusMixer Trainium2 kernel.

Computes, for x [B, 4096] and Q [32, 128, 128]:
    z[b, n, :] = relu(x[b, n*128:(n+1)*128] @ Q[n])
    y = z.reshape(B, 4096);  y /= max(||y||_row, 1e-12)

Strategy: data-parallel over 8 NeuronCores (2048 batch rows each).
The contraction dim (d within a block) must live on SBUF partitions for
the tensor engine, so the host pre-tiles x into the exact per-DMA-step
SBUF image xtt[t, d, n, b] = x_shard[t*NB + b, n*128 + d]: every DMA is
then a fully contiguous block (32KB per partition).  The kernel runs the
32 block matmuls with Q_n stationary, relu's PSUM->SBUF on the scalar
engine, squares (bf16) on the scalar engine, reduces the squares across
all 4096 features with a ones-vector matmul accumulated in PSUM,
broadcasts 1/norm across partitions with a K=1 matmul, scales on the
vector engine, and streams the result out in the same tiled layout,
which the host un-tiles.
"""

import numpy as np

import concourse.bass as bass
import concourse.tile as tile
from concourse import bacc, mybir
from concourse.bass_utils import run_bass_kernel_spmd

N_CORES = 8
BATCH = 16384
D = 4096
NBLK = 32
BD = 128
B_CORE = BATCH // N_CORES  # 2048
NB = 256  # batch columns per pipeline step
NSTEPS = B_CORE // NB
EPS2 = 1e-24  # eps**2 so that max(sqrt(s), eps) == sqrt(max(s, eps**2))

FP32 = mybir.dt.float32
BF16 = mybir.dt.bfloat16


# input/matmul/io precision mode:
#   "f32"  - fp32 storage, exact fp32 matmul (4x slower PE)
#   "f32r" - fp32 storage, reduced-precision full-rate matmul
#   "bf16" - bf16 storage+matmul, halves input DMA
#   "fp16" - fp16 storage+matmul AND fp16 z/output: halves all DMA with
#            11-bit mantissa (~8x more accurate than bf16)
IN_MODE = "fp16"


def build_kernel(
    nsteps: int = NSTEPS,
    nb: int = NB,
    repeat: int = 1,
    in_mode: str | None = None,
    probe: str | None = None,  # None | "dma" | "compute" | "mm" | "mmsq"
):
    mode = in_mode or IN_MODE
    mm_f32r = mode == "f32r"
    """Builds + compiles the per-core Bass module. All 8 cores run the
    same NEFF on their own batch shard. repeat>1 wraps the whole pipeline
    in a hardware loop re-doing identical work — used only for timing
    (wall-clock slope between repeat=1 and repeat=K cancels transfer and
    dispatch overhead)."""
    nc = bacc.Bacc(
        "TRN2",
        target_bir_lowering=False,
        debug=False,
        enable_asserts=False,
        num_devices=N_CORES,
    )
    in_dt = {
        "fp16": mybir.dt.float16,
        "bf16": BF16,
        "f32r": mybir.dt.float32r,
        "f32": FP32,
    }[mode]
    half = mode in ("fp16", "bf16")
    z_dt = in_dt if half else FP32
    sq_dt = in_dt if half else BF16
    out_dt = in_dt if half else FP32
    xt = nc.dram_tensor(
        "xt", [nsteps, BD, NBLK * nb], in_dt, kind="ExternalInput"
    ).ap()
    q = nc.dram_tensor("q", [BD, NBLK * BD], in_dt, kind="ExternalInput").ap()
    y = nc.dram_tensor(
        "y", [nsteps, BD, NBLK * nb], out_dt, kind="ExternalOutput"
    ).ap()

    with tile.TileContext(nc) as tc:
        with (
            tc.tile_pool(name="qpool", bufs=1) as qpool,
            tc.tile_pool(name="xpool", bufs=4) as xpool,
            tc.tile_pool(name="zpool", bufs=3) as zpool,
            tc.tile_pool(name="sqpool", bufs=3) as sqpool,
            tc.tile_pool(name="consts", bufs=1) as consts,
            tc.tile_pool(name="npool", bufs=2) as npool,
            tc.tile_pool(name="mm_psum", bufs=2, space="PSUM") as mm_psum,
            tc.tile_pool(name="s_psum", bufs=2, space="PSUM") as s_psum,
            tc.tile_pool(name="b_psum", bufs=2, space="PSUM") as b_psum,
        ):
            ones_col = consts.tile([BD, 1], sq_dt)  # lhsT of the sum-reduce matmul
            nc.vector.memset(ones_col[:], 1.0)
            ones_row = consts.tile([1, BD], FP32)  # lhsT of the broadcast matmul
            nc.vector.memset(ones_row[:], 1.0)
            eps_c = consts.tile([1, 1], FP32)  # sqrt bias = eps^2
            nc.vector.memset(eps_c[:], EPS2)

            def rep_body():
                # Q in SBUF: partition = d, free = (n, e)
                q_sb = qpool.tile([BD, NBLK, BD], in_dt)
                nc.sync.dma_start(q_sb[:], q.rearrange("d (n e) -> d n e", e=BD))

                compute_like = probe in ("compute", "mm", "mmsq")
                x_held = None
                for t in range(nsteps):
                    if compute_like:
                        if x_held is None:
                            x_held = xpool.tile([BD, NBLK, nb], in_dt)
                            nc.sync.dma_start(
                                x_held[:], xt[0].rearrange("d (n b) -> d n b", b=nb)
                            )
                        x_sb = x_held
                    else:
                        x_sb = xpool.tile([BD, NBLK, nb], in_dt)
                        xt_t = xt[t].rearrange("d (n b) -> d n b", b=nb)
                        hn = NBLK // 2
                        nc.sync.dma_start(x_sb[:, :hn, :], xt_t[:, :hn, :])
                        nc.sync.dma_start(x_sb[:, hn:, :], xt_t[:, hn:, :])
                    if probe == "dma":
                        # pure-DMA probe: stream back out the input tile
                        nc.sync.dma_start(
                            y[t].rearrange("d (n b) -> d n b", b=nb),
                            x_sb[:].bitcast(out_dt),
                        )
                        continue
                    z_sb = zpool.tile([BD, NBLK, nb], z_dt)

                    # block matmuls, four per two-bank PSUM tile so the relu
                    # reads 1024-wide
                    for jp in range(NBLK // 4):
                        z_ps = mm_psum.tile([BD, 4, nb], FP32)
                        for h in range(4):
                            n = 4 * jp + h
                            nc.tensor.matmul(
                                z_ps[:, h, :],
                                q_sb[:, n, :],
                                x_sb[:, n, :],
                                start=True,
                                stop=True,
                            )
                        nc.scalar.activation(
                            z_sb[:, 4 * jp : 4 * jp + 4, :],
                            z_ps[:],
                            mybir.ActivationFunctionType.Relu,
                        )

                    if probe == "mm":
                        nc.sync.dma_start(
                            y[t].rearrange("d (n b) -> d n b", b=nb)[:, :1, :],
                            z_sb[:, :1, :],
                        )
                        continue

                    # squared relu (bf16) -> ones-matmul accumulates sum over
                    # all 4096 features into s_ps[1, nb]; squares alternate
                    # between ScalarE and VectorE to balance engine load
                    if probe != "mmsq":
                        s_ps = s_psum.tile([1, nb], FP32)
                    for jc in range(NBLK // 4):
                        sq = sqpool.tile([BD, 4, nb], sq_dt)
                        zc = z_sb[:, 4 * jc : 4 * jc + 4, :]
                        if jc % 4 == 0:
                            nc.scalar.activation(
                                sq[:], zc, mybir.ActivationFunctionType.Square
                            )
                        else:
                            nc.vector.tensor_mul(sq[:], zc, zc)
                        if probe == "mmsq":
                            nc.sync.dma_start(
                                y[t].rearrange("d (n b) -> d n b", b=nb)[
                                    :, jc : jc + 1, :
                                ].bitcast(BF16)[:, :, :nb],
                                sq[:, :1, :],
                            )
                            continue
                        for h in range(4):
                            n = 4 * jc + h
                            nc.tensor.matmul(
                                s_ps[:],
                                ones_col[:],
                                sq[:, h, :],
                                start=(n == 0),
                                stop=(n == NBLK - 1),
                            )
                    if probe == "mmsq":
                        continue

                    # recip = 1 / sqrt(s + eps^2)  (== 1/max(sqrt(s), eps)
                    # to working precision since s >= 0)
                    nrm = npool.tile([1, nb], FP32)
                    nc.scalar.activation(
                        nrm[:], s_ps[:], mybir.ActivationFunctionType.Sqrt,
                        bias=eps_c[:],
                    )
                    recip = npool.tile([1, nb], FP32)
                    nc.vector.reciprocal(recip[:], nrm[:])

                    # broadcast recip across the 128 partitions via a K=1 matmul
                    bc_ps = b_psum.tile([BD, nb], FP32)
                    nc.tensor.matmul(
                        bc_ps[:], ones_row[:], recip[:], start=True, stop=True
                    )

                    # y = relu(z) * recip  (in place), in 4 chunks so each
                    # output-DMA chunk can start as soon as its scale is done
                    if half:
                        bc_sb = npool.tile([BD, nb], z_dt, tag="bc_sb")
                        nc.scalar.copy(bc_sb[:], bc_ps[:])
                        bc_op = bc_sb
                    else:
                        bc_op = bc_ps
                    ch = NBLK // 4
                    y_t = y[t].rearrange("d (n b) -> d n b", b=nb)
                    for c in range(4):
                        zc4 = z_sb[:, c * ch : (c + 1) * ch, :]
                        nc.vector.tensor_mul(
                            zc4,
                            zc4,
                            bc_op[:, None, :].broadcast_to([BD, ch, nb]),
                        )
                        if probe == "compute":
                            if c == 0:
                                nc.sync.dma_start(
                                    y_t[:, :1, :], z_sb[:, :1, :]
                                )
                        else:
                            nc.sync.dma_start(
                                y_t[:, c * ch : (c + 1) * ch, :], zc4
                            )

            hint = () if probe == "dma" else (mybir.EngineType.PE,)
            if repeat == 1:
                rep_body()
            else:
                with tc.For_i(0, repeat, 1, hint_engines=hint):
                    rep_body()

    nc.compile()
    return nc


_NC_CACHE: dict = {}


def _get_nc():
    if "nc" not in _NC_CACHE:
        _NC_CACHE["nc"] = build_kernel()
    return _NC_CACHE["nc"]


def shard_inputs(x: np.ndarray, Q: np.ndarray, in_mode: str | None = None) -> list[dict]:
    """Per-core input maps in the pre-tiled DMA-friendly layout:
    xtt[t, d, n*nb + b] = x_shard[t*NB + b, n*128 + d]."""
    mode = in_mode or IN_MODE
    if mode == "bf16":
        import ml_dtypes

        host_dt = np.dtype(ml_dtypes.bfloat16)
    elif mode == "fp16":
        host_dt = np.dtype(np.float16)
    else:
        host_dt = np.dtype(np.float32)
    x = np.asarray(x, dtype=np.float32)
    Q = np.asarray(Q, dtype=np.float32)
    qh = np.ascontiguousarray(
        Q.transpose(1, 0, 2).astype(host_dt)
    ).reshape(BD, NBLK * BD)
    xs = x.reshape(N_CORES, NSTEPS, NB, NBLK, BD)  # [c, t, b, n, d]
    in_maps = []
    for c in range(N_CORES):
        xtt = np.ascontiguousarray(
            xs[c].transpose(0, 3, 2, 1).astype(host_dt)
        )  # [t, d, n, b]
        in_maps.append(
            {"xt": xtt.reshape(NSTEPS, BD, NBLK * NB), "q": qh}
        )
    return in_maps


def unshard_output(results: list[dict]) -> np.ndarray:
    out = np.empty((N_CORES, NSTEPS, NB, NBLK, BD), dtype=np.float32)
    for c in range(N_CORES):
        ytt = results[c]["y"].reshape(NSTEPS, BD, NBLK, NB)
        out[c] = ytt.transpose(0, 3, 2, 1).astype(np.float32)  # -> [t, b, n, d]
    return out.reshape(BATCH, D)


def kernel(x, Q) -> np.ndarray:
    nc = _get_nc()
    in_maps = shard_inputs(x, Q)
    res = run_bass_kernel_spmd(nc, in_maps, core_ids=list(range(N_CORES)))
    return unshard_output(res.results)


# revision 46
# speedup vs baseline: 1.0286x; 1.0286x over previous
"""BlockRadiusMixer Trainium2 kernel.

Computes, for x [B, 4096] and Q [32, 128, 128]:
    z[b, n, :] = relu(x[b, n*128:(n+1)*128] @ Q[n])
    y = z.reshape(B, 4096);  y /= max(||y||_row, 1e-12)

Strategy: data-parallel over 8 NeuronCores (2048 batch rows each).
The contraction dim (d within a block) must live on SBUF partitions for
the tensor engine, so the host pre-tiles x into the exact per-DMA-step
SBUF image xtt[t, d, n, b] = x_shard[t*NB + b, n*128 + d]: every DMA is
then a fully contiguous block (32KB per partition).  The kernel runs the
32 block matmuls with Q_n stationary, relu's PSUM->SBUF on the scalar
engine, squares (bf16) on the scalar engine, reduces the squares across
all 4096 features with a ones-vector matmul accumulated in PSUM,
broadcasts 1/norm across partitions with a K=1 matmul, scales on the
vector engine, and streams the result out in the same tiled layout,
which the host un-tiles.
"""

import numpy as np

import concourse.bass as bass
import concourse.tile as tile
from concourse import bacc, mybir
from concourse.bass_utils import run_bass_kernel_spmd

N_CORES = 8
BATCH = 16384
D = 4096
NBLK = 32
BD = 128
B_CORE = BATCH // N_CORES  # 2048
NB = 256  # batch columns per pipeline step
NSTEPS = B_CORE // NB
EPS2 = 1e-24  # eps**2 so that max(sqrt(s), eps) == sqrt(max(s, eps**2))

FP32 = mybir.dt.float32
BF16 = mybir.dt.bfloat16


# input/matmul/io precision mode:
#   "f32"  - fp32 storage, exact fp32 matmul (4x slower PE)
#   "f32r" - fp32 storage, reduced-precision full-rate matmul
#   "bf16" - bf16 storage+matmul, halves input DMA
#   "fp16" - fp16 storage+matmul AND fp16 z/output: halves all DMA with
#            11-bit mantissa (~8x more accurate than bf16)
IN_MODE = "fp16"
SPLIT_IN = True
SQ_ALL_DVE = False
OUT_SWDGE = False
OUT_ALT_RING = False
OUT_CHUNKS = 2


def build_kernel(
    nsteps: int = NSTEPS,
    nb: int = NB,
    repeat: int = 1,
    in_mode: str | None = None,
    probe: str | None = None,  # None | "dma" | "compute" | "mm" | "mmsq"
):
    mode = in_mode or IN_MODE
    mm_f32r = mode == "f32r"
    """Builds + compiles the per-core Bass module. All 8 cores run the
    same NEFF on their own batch shard. repeat>1 wraps the whole pipeline
    in a hardware loop re-doing identical work — used only for timing
    (wall-clock slope between repeat=1 and repeat=K cancels transfer and
    dispatch overhead)."""
    nc = bacc.Bacc(
        "TRN2",
        target_bir_lowering=False,
        debug=False,
        enable_asserts=False,
        num_devices=N_CORES,
    )
    in_dt = {
        "fp16": mybir.dt.float16,
        "bf16": BF16,
        "f32r": mybir.dt.float32r,
        "f32": FP32,
    }[mode]
    half = mode in ("fp16", "bf16")
    z_dt = in_dt if half else FP32
    sq_dt = in_dt if half else BF16
    out_dt = in_dt if half else FP32
    xt = nc.dram_tensor(
        "xt", [nsteps, BD, NBLK * nb], in_dt, kind="ExternalInput"
    ).ap()
    q = nc.dram_tensor("q", [BD, NBLK * BD], in_dt, kind="ExternalInput").ap()
    y = nc.dram_tensor(
        "y", [nsteps, BD, NBLK * nb], out_dt, kind="ExternalOutput"
    ).ap()

    xbufs = 4 if nb <= 256 else 2
    zbufs = 3 if nb <= 256 else 2
    with tile.TileContext(nc) as tc:
        with (
            tc.tile_pool(name="qpool", bufs=1) as qpool,
            tc.tile_pool(name="xpool", bufs=xbufs) as xpool,
            tc.tile_pool(name="zpool", bufs=zbufs) as zpool,
            tc.tile_pool(name="sqpool", bufs=3) as sqpool,
            tc.tile_pool(name="consts", bufs=1) as consts,
            tc.tile_pool(name="npool", bufs=2) as npool,
            tc.tile_pool(name="mm_psum", bufs=2, space="PSUM") as mm_psum,
            tc.tile_pool(name="s_psum", bufs=2, space="PSUM") as s_psum,
            tc.tile_pool(name="b_psum", bufs=2, space="PSUM") as b_psum,
        ):
            ones_col = consts.tile([BD, 1], sq_dt)  # lhsT of the sum-reduce matmul
            nc.vector.memset(ones_col[:], 1.0)
            ones_row = consts.tile([1, BD], FP32)  # lhsT of the broadcast matmul
            nc.vector.memset(ones_row[:], 1.0)
            eps_c = consts.tile([1, 1], FP32)  # sqrt bias = eps^2
            nc.vector.memset(eps_c[:], EPS2)

            def rep_body():
                # Q in SBUF: partition = d, free = (n, e)
                q_sb = qpool.tile([BD, NBLK, BD], in_dt)
                nc.sync.dma_start(q_sb[:], q.rearrange("d (n e) -> d n e", e=BD))

                compute_like = probe in ("compute", "mm", "mmsq")
                x_held = None
                for t in range(nsteps):
                    if compute_like:
                        if x_held is None:
                            x_held = xpool.tile([BD, NBLK, nb], in_dt)
                            nc.sync.dma_start(
                                x_held[:], xt[0].rearrange("d (n b) -> d n b", b=nb)
                            )
                        x_sb = x_held
                    else:
                        x_sb = xpool.tile([BD, NBLK, nb], in_dt)
                        xt_t = xt[t].rearrange("d (n b) -> d n b", b=nb)
                        if SPLIT_IN:
                            hn = NBLK // 2
                            nc.sync.dma_start(x_sb[:, :hn, :], xt_t[:, :hn, :])
                            nc.sync.dma_start(x_sb[:, hn:, :], xt_t[:, hn:, :])
                        else:
                            nc.sync.dma_start(x_sb[:], xt_t)
                    if probe == "dma":
                        # pure-DMA probe: stream back out the input tile
                        nc.sync.dma_start(
                            y[t].rearrange("d (n b) -> d n b", b=nb),
                            x_sb[:].bitcast(out_dt),
                        )
                        continue
                    z_sb = zpool.tile([BD, NBLK, nb], z_dt)

                    # block matmuls, grouped so each two-bank PSUM tile gives
                    # the relu a ~1024-wide read
                    g = max(1, 1024 // nb)
                    for jp in range(NBLK // g):
                        z_ps = mm_psum.tile([BD, g, nb], FP32)
                        for h in range(g):
                            n = g * jp + h
                            nc.tensor.matmul(
                                z_ps[:, h, :],
                                q_sb[:, n, :],
                                x_sb[:, n, :],
                                start=True,
                                stop=True,
                            )
                        nc.scalar.activation(
                            z_sb[:, g * jp : g * jp + g, :],
                            z_ps[:],
                            mybir.ActivationFunctionType.Relu,
                        )

                    if probe == "mm":
                        nc.sync.dma_start(
                            y[t].rearrange("d (n b) -> d n b", b=nb)[:, :1, :],
                            z_sb[:, :1, :],
                        )
                        continue

                    # squared relu (bf16) -> ones-matmul accumulates sum over
                    # all 4096 features into s_ps[1, nb]; squares alternate
                    # between ScalarE and VectorE to balance engine load
                    if probe != "mmsq":
                        s_ps = s_psum.tile([1, nb], FP32)
                    # (cg blocks per square chunk)
                    cg = max(1, 1024 // nb)
                    for jc in range(NBLK // cg):
                        sq = sqpool.tile([BD, cg, nb], sq_dt)
                        zc = z_sb[:, cg * jc : cg * jc + cg, :]
                        if jc % 4 == 0 and not SQ_ALL_DVE:
                            nc.scalar.activation(
                                sq[:], zc, mybir.ActivationFunctionType.Square
                            )
                        else:
                            nc.vector.tensor_mul(sq[:], zc, zc)
                        if probe == "mmsq":
                            nc.sync.dma_start(
                                y[t].rearrange("d (n b) -> d n b", b=nb)[
                                    :, jc : jc + 1, :
                                ].bitcast(BF16)[:, :, :nb],
                                sq[:, :1, :],
                            )
                            continue
                        for h in range(cg):
                            n = cg * jc + h
                            nc.tensor.matmul(
                                s_ps[:],
                                ones_col[:],
                                sq[:, h, :],
                                start=(n == 0),
                                stop=(n == NBLK - 1),
                            )
                    if probe == "mmsq":
                        continue

                    # recip = 1 / sqrt(s + eps^2)  (== 1/max(sqrt(s), eps)
                    # to working precision since s >= 0)
                    nrm = npool.tile([1, nb], FP32)
                    nc.scalar.activation(
                        nrm[:], s_ps[:], mybir.ActivationFunctionType.Sqrt,
                        bias=eps_c[:],
                    )
                    recip = npool.tile([1, nb], FP32)
                    nc.vector.reciprocal(recip[:], nrm[:])

                    # broadcast recip across the 128 partitions via a K=1 matmul
                    bc_ps = b_psum.tile([BD, nb], FP32)
                    nc.tensor.matmul(
                        bc_ps[:], ones_row[:], recip[:], start=True, stop=True
                    )

                    # y = relu(z) * recip  (in place), in 4 chunks so each
                    # output-DMA chunk can start as soon as its scale is done
                    if half:
                        bc_sb = npool.tile([BD, nb], z_dt, tag="bc_sb")
                        nc.scalar.copy(bc_sb[:], bc_ps[:])
                        bc_op = bc_sb
                    else:
                        bc_op = bc_ps
                    nch = OUT_CHUNKS
                    ch = NBLK // nch
                    y_t = y[t].rearrange("d (n b) -> d n b", b=nb)
                    for c in range(nch):
                        zc4 = z_sb[:, c * ch : (c + 1) * ch, :]
                        nc.vector.tensor_mul(
                            zc4,
                            zc4,
                            bc_op[:, None, :].broadcast_to([BD, ch, nb]),
                        )
                        if probe == "compute":
                            if c == 0:
                                nc.sync.dma_start(
                                    y_t[:, :1, :], z_sb[:, :1, :]
                                )
                        else:
                            if OUT_SWDGE:
                                eng = nc.gpsimd
                            elif OUT_ALT_RING and c % 2 == 1:
                                eng = nc.scalar
                            else:
                                eng = nc.sync
                            eng.dma_start(
                                y_t[:, c * ch : (c + 1) * ch, :], zc4
                            )

            hint = () if probe == "dma" else (mybir.EngineType.PE,)
            if repeat == 1:
                rep_body()
            else:
                with tc.For_i(0, repeat, 1, hint_engines=hint):
                    rep_body()

    nc.compile()
    return nc


_NC_CACHE: dict = {}


def _get_nc():
    if "nc" not in _NC_CACHE:
        _NC_CACHE["nc"] = build_kernel()
    return _NC_CACHE["nc"]


def shard_inputs(x: np.ndarray, Q: np.ndarray, in_mode: str | None = None) -> list[dict]:
    """Per-core input maps in the pre-tiled DMA-friendly layout:
    xtt[t, d, n*nb + b] = x_shard[t*NB + b, n*128 + d]."""
    mode = in_mode or IN_MODE
    if mode == "bf16":
        import ml_dtypes

        host_dt = np.dtype(ml_dtypes.bfloat16)
    elif mode == "fp16":
        host_dt = np.dtype(np.float16)
    else:
        host_dt = np.dtype(np.float32)
    x = np.asarray(x, dtype=np.float32)
    Q = np.asarray(Q, dtype=np.float32)
    qh = np.ascontiguousarray(
        Q.transpose(1, 0, 2).astype(host_dt)
    ).reshape(BD, NBLK * BD)
    xs = x.reshape(N_CORES, NSTEPS, NB, NBLK, BD)  # [c, t, b, n, d]
    in_maps = []
    for c in range(N_CORES):
        xtt = np.ascontiguousarray(
            xs[c].transpose(0, 3, 2, 1).astype(host_dt)
        )  # [t, d, n, b]
        in_maps.append(
            {"xt": xtt.reshape(NSTEPS, BD, NBLK * NB), "q": qh}
        )
    return in_maps


def unshard_output(results: list[dict]) -> np.ndarray:
    out = np.empty((N_CORES, NSTEPS, NB, NBLK, BD), dtype=np.float32)
    for c in range(N_CORES):
        ytt = results[c]["y"].reshape(NSTEPS, BD, NBLK, NB)
        out[c] = ytt.transpose(0, 3, 2, 1).astype(np.float32)  # -> [t, b, n, d]
    return out.reshape(BATCH, D)


def kernel(x, Q) -> np.ndarray:
    nc = _get_nc()
    in_maps = shard_inputs(x, Q)
    res = run_bass_kernel_spmd(nc, in_maps, core_ids=list(range(N_CORES)))
    return unshard_output(res.results)


# revision 49
# speedup vs baseline: 1.0450x; 1.0159x over previous
"""BlockRadiusMixer Trainium2 kernel.

Computes, for x [B, 4096] and Q [32, 128, 128]:
    z[b, n, :] = relu(x[b, n*128:(n+1)*128] @ Q[n])
    y = z.reshape(B, 4096);  y /= max(||y||_row, 1e-12)

Strategy: data-parallel over 8 NeuronCores (2048 batch rows each).
The contraction dim (d within a block) must live on SBUF partitions for
the tensor engine, so the host pre-tiles x into the exact per-DMA-step
SBUF image xtt[t, d, n, b] = x_shard[t*NB + b, n*128 + d]: every DMA is
then a fully contiguous block.  Data moves as fp16 (11-bit mantissa;
all value ranges here fit comfortably), which halves HBM traffic — the
binding resource — and runs the PE at full rate.  Per batch tile the
kernel runs the 32 block matmuls with Q_n stationary (fp32 PSUM accum),
relu's PSUM->SBUF on the scalar engine, squares on ScalarE/VectorE,
reduces the squares across all 4096 features with a ones-vector matmul
accumulated in PSUM, takes 1/sqrt(s+eps^2) (ScalarE sqrt + VectorE
reciprocal), broadcasts it across partitions with a K=1 matmul, scales
on the vector engine in chunks so output DMA starts early, and streams
the result out in the same tiled layout, which the host un-tiles.

Measured on 8 axon-tunneled trn2 cores: ~132 us/execution (steady-state
slope, all 8 cores active), absmax-relative error 9.1e-4 vs the fp32
reference (fp16 input/output rounding dominated).  HBM floor for the
fp16 traffic (34MB/core at ~347GB/s) is ~100 us.
"""

import numpy as np

import concourse.bass as bass
import concourse.tile as tile
from concourse import bacc, mybir
from concourse.bass_utils import run_bass_kernel_spmd

N_CORES = 8
BATCH = 16384
D = 4096
NBLK = 32
BD = 128
B_CORE = BATCH // N_CORES  # 2048
NB = 256  # batch columns per pipeline step
NSTEPS = B_CORE // NB
EPS2 = 1e-24  # eps**2 so that max(sqrt(s), eps) == sqrt(max(s, eps**2))

FP32 = mybir.dt.float32
BF16 = mybir.dt.bfloat16


# input/matmul/io precision mode:
#   "f32"  - fp32 storage, exact fp32 matmul (4x slower PE)
#   "f32r" - fp32 storage, reduced-precision full-rate matmul
#   "bf16" - bf16 storage+matmul, halves input DMA
#   "fp16" - fp16 storage+matmul AND fp16 z/output: halves all DMA with
#            11-bit mantissa (~8x more accurate than bf16)
IN_MODE = "fp16"
SPLIT_IN = 2
SQ_ALL_DVE = False
OUT_SWDGE = False
OUT_ALT_RING = False
OUT_CHUNKS = 2
PSUM_GROUP = 1024
MM_PSUM_BUFS = 2


def build_kernel(
    nsteps: int = NSTEPS,
    nb: int = NB,
    repeat: int = 1,
    in_mode: str | None = None,
    probe: str | None = None,  # None | "dma" | "compute" | "mm" | "mmsq"
):
    """Builds + compiles the per-core Bass module. All 8 cores run the
    same NEFF on their own batch shard. repeat>1 wraps the whole pipeline
    in a hardware loop re-doing identical work — used only for timing
    (wall-clock slope between repeat=1 and repeat=K cancels transfer and
    dispatch overhead)."""
    mode = in_mode or IN_MODE
    nc = bacc.Bacc(
        "TRN2",
        target_bir_lowering=False,
        debug=False,
        enable_asserts=False,
        num_devices=N_CORES,
    )
    in_dt = {
        "fp16": mybir.dt.float16,
        "bf16": BF16,
        "f32r": mybir.dt.float32r,
        "f32": FP32,
    }[mode]
    half = mode in ("fp16", "bf16")
    z_dt = in_dt if half else FP32
    sq_dt = in_dt if half else BF16
    out_dt = in_dt if half else FP32
    xt = nc.dram_tensor(
        "xt", [nsteps, BD, NBLK * nb], in_dt, kind="ExternalInput"
    ).ap()
    q = nc.dram_tensor("q", [BD, NBLK * BD], in_dt, kind="ExternalInput").ap()
    y = nc.dram_tensor(
        "y", [nsteps, BD, NBLK * nb], out_dt, kind="ExternalOutput"
    ).ap()

    xbufs = 4 if nb <= 256 else 2
    zbufs = 3 if nb <= 256 else 2
    with tile.TileContext(nc) as tc:
        with (
            tc.tile_pool(name="qpool", bufs=1) as qpool,
            tc.tile_pool(name="xpool", bufs=xbufs) as xpool,
            tc.tile_pool(name="zpool", bufs=zbufs) as zpool,
            tc.tile_pool(name="sqpool", bufs=3) as sqpool,
            tc.tile_pool(name="consts", bufs=1) as consts,
            tc.tile_pool(name="npool", bufs=2) as npool,
            tc.tile_pool(name="mm_psum", bufs=MM_PSUM_BUFS, space="PSUM") as mm_psum,
            tc.tile_pool(name="s_psum", bufs=2, space="PSUM") as s_psum,
            tc.tile_pool(name="b_psum", bufs=2, space="PSUM") as b_psum,
        ):
            ones_col = consts.tile([BD, 1], sq_dt)  # lhsT of the sum-reduce matmul
            nc.vector.memset(ones_col[:], 1.0)
            ones_row = consts.tile([1, BD], FP32)  # lhsT of the broadcast matmul
            nc.vector.memset(ones_row[:], 1.0)
            eps_c = consts.tile([1, 1], FP32)  # sqrt bias = eps^2
            nc.vector.memset(eps_c[:], EPS2)

            def rep_body():
                # Q in SBUF: partition = d, free = (n, e)
                q_sb = qpool.tile([BD, NBLK, BD], in_dt)
                nc.sync.dma_start(q_sb[:], q.rearrange("d (n e) -> d n e", e=BD))

                compute_like = probe in ("compute", "mm", "mmsq")
                x_held = None
                for t in range(nsteps):
                    if compute_like:
                        if x_held is None:
                            x_held = xpool.tile([BD, NBLK, nb], in_dt)
                            nc.sync.dma_start(
                                x_held[:], xt[0].rearrange("d (n b) -> d n b", b=nb)
                            )
                        x_sb = x_held
                    else:
                        x_sb = xpool.tile([BD, NBLK, nb], in_dt)
                        xt_t = xt[t].rearrange("d (n b) -> d n b", b=nb)
                        if SPLIT_IN:
                            nsp = SPLIT_IN if SPLIT_IN > 1 else 2
                            hn = NBLK // nsp
                            for sp in range(nsp):
                                nc.sync.dma_start(
                                    x_sb[:, sp * hn : (sp + 1) * hn, :],
                                    xt_t[:, sp * hn : (sp + 1) * hn, :],
                                )
                        else:
                            nc.sync.dma_start(x_sb[:], xt_t)
                    if probe == "dma":
                        # pure-DMA probe: stream back out the input tile
                        nc.sync.dma_start(
                            y[t].rearrange("d (n b) -> d n b", b=nb),
                            x_sb[:].bitcast(out_dt),
                        )
                        continue
                    z_sb = zpool.tile([BD, NBLK, nb], z_dt)

                    # block matmuls, grouped so each two-bank PSUM tile gives
                    # the relu a ~1024-wide read
                    g = max(1, PSUM_GROUP // nb)
                    for jp in range(NBLK // g):
                        z_ps = mm_psum.tile([BD, g, nb], FP32)
                        for h in range(g):
                            n = g * jp + h
                            nc.tensor.matmul(
                                z_ps[:, h, :],
                                q_sb[:, n, :],
                                x_sb[:, n, :],
                                start=True,
                                stop=True,
                            )
                        nc.scalar.activation(
                            z_sb[:, g * jp : g * jp + g, :],
                            z_ps[:],
                            mybir.ActivationFunctionType.Relu,
                        )

                    if probe == "mm":
                        nc.sync.dma_start(
                            y[t].rearrange("d (n b) -> d n b", b=nb)[:, :1, :],
                            z_sb[:, :1, :],
                        )
                        continue

                    # squared relu (bf16) -> ones-matmul accumulates sum over
                    # all 4096 features into s_ps[1, nb]; squares alternate
                    # between ScalarE and VectorE to balance engine load
                    if probe != "mmsq":
                        s_ps = s_psum.tile([1, nb], FP32)
                    # (cg blocks per square chunk)
                    cg = max(1, 1024 // nb)
                    for jc in range(NBLK // cg):
                        sq = sqpool.tile([BD, cg, nb], sq_dt)
                        zc = z_sb[:, cg * jc : cg * jc + cg, :]
                        if jc % 4 == 0 and not SQ_ALL_DVE:
                            nc.scalar.activation(
                                sq[:], zc, mybir.ActivationFunctionType.Square
                            )
                        else:
                            nc.vector.tensor_mul(sq[:], zc, zc)
                        if probe == "mmsq":
                            nc.sync.dma_start(
                                y[t].rearrange("d (n b) -> d n b", b=nb)[
                                    :, jc : jc + 1, :
                                ].bitcast(BF16)[:, :, :nb],
                                sq[:, :1, :],
                            )
                            continue
                        for h in range(cg):
                            n = cg * jc + h
                            nc.tensor.matmul(
                                s_ps[:],
                                ones_col[:],
                                sq[:, h, :],
                                start=(n == 0),
                                stop=(n == NBLK - 1),
                            )
                    if probe == "mmsq":
                        continue

                    # recip = 1 / sqrt(s + eps^2)  (== 1/max(sqrt(s), eps)
                    # to working precision since s >= 0)
                    nrm = npool.tile([1, nb], FP32)
                    nc.scalar.activation(
                        nrm[:], s_ps[:], mybir.ActivationFunctionType.Sqrt,
                        bias=eps_c[:],
                    )
                    recip = npool.tile([1, nb], FP32)
                    nc.vector.reciprocal(recip[:], nrm[:])

                    # broadcast recip across the 128 partitions via a K=1 matmul
                    bc_ps = b_psum.tile([BD, nb], FP32)
                    nc.tensor.matmul(
                        bc_ps[:], ones_row[:], recip[:], start=True, stop=True
                    )

                    # y = relu(z) * recip  (in place), chunked so each
                    # output-DMA chunk can start as soon as its scale is done
                    if half:
                        bc_sb = npool.tile([BD, nb], z_dt, tag="bc_sb")
                        nc.scalar.copy(bc_sb[:], bc_ps[:])
                        bc_op = bc_sb
                    else:
                        bc_op = bc_ps
                    nch = OUT_CHUNKS
                    ch = NBLK // nch
                    y_t = y[t].rearrange("d (n b) -> d n b", b=nb)
                    for c in range(nch):
                        zc4 = z_sb[:, c * ch : (c + 1) * ch, :]
                        nc.vector.tensor_mul(
                            zc4,
                            zc4,
                            bc_op[:, None, :].broadcast_to([BD, ch, nb]),
                        )
                        if probe == "compute":
                            if c == 0:
                                nc.sync.dma_start(
                                    y_t[:, :1, :], z_sb[:, :1, :]
                                )
                        else:
                            if OUT_SWDGE:
                                eng = nc.gpsimd
                            elif OUT_ALT_RING and c % 2 == 1:
                                eng = nc.scalar
                            else:
                                eng = nc.sync
                            eng.dma_start(
                                y_t[:, c * ch : (c + 1) * ch, :], zc4
                            )

            hint = () if probe == "dma" else (mybir.EngineType.PE,)
            if repeat == 1:
                rep_body()
            else:
                with tc.For_i(0, repeat, 1, hint_engines=hint):
                    rep_body()

    nc.compile()
    return nc


_NC_CACHE: dict = {}


def _get_nc():
    if "nc" not in _NC_CACHE:
        _NC_CACHE["nc"] = build_kernel()
    return _NC_CACHE["nc"]


def shard_inputs(x: np.ndarray, Q: np.ndarray, in_mode: str | None = None) -> list[dict]:
    """Per-core input maps in the pre-tiled DMA-friendly layout:
    xtt[t, d, n*nb + b] = x_shard[t*NB + b, n*128 + d]."""
    mode = in_mode or IN_MODE
    if mode == "bf16":
        import ml_dtypes

        host_dt = np.dtype(ml_dtypes.bfloat16)
    elif mode == "fp16":
        host_dt = np.dtype(np.float16)
    else:
        host_dt = np.dtype(np.float32)
    x = np.asarray(x, dtype=np.float32)
    Q = np.asarray(Q, dtype=np.float32)
    qh = np.ascontiguousarray(
        Q.transpose(1, 0, 2).astype(host_dt)
    ).reshape(BD, NBLK * BD)
    xs = x.reshape(N_CORES, NSTEPS, NB, NBLK, BD)  # [c, t, b, n, d]
    in_maps = []
    for c in range(N_CORES):
        xtt = np.ascontiguousarray(
            xs[c].transpose(0, 3, 2, 1).astype(host_dt)
        )  # [t, d, n, b]
        in_maps.append(
            {"xt": xtt.reshape(NSTEPS, BD, NBLK * NB), "q": qh}
        )
    return in_maps


def unshard_output(results: list[dict]) -> np.ndarray:
    out = np.empty((N_CORES, NSTEPS, NB, NBLK, BD), dtype=np.float32)
    for c in range(N_CORES):
        ytt = results[c]["y"].reshape(NSTEPS, BD, NBLK, NB)
        out[c] = ytt.transpose(0, 3, 2, 1).astype(np.float32)  # -> [t, b, n, d]
    return out.reshape(BATCH, D)


def kernel(x, Q) -> np.ndarray:
    nc = _get_nc()
    in_maps = shard_inputs(x, Q)
    res = run_bass_kernel_spmd(nc, in_maps, core_ids=list(range(N_CORES)))
    return unshard_output(res.results)
